# revision 12
# baseline (speedup 1.0000x reference)
"""Trainium2 Bass kernel for nn_Conv2d_86191403696259.

v2c: 2-pass (dh,dw)-folded matmul (256 MMs vs baseline 384) with 32-aligned
partition groups, the 9th (dh=2,dw=2) group DMA'd into pass-A gap partitions
("gap rider"), fp16 output stores with 2-row 8KB descriptors, and host-prepped
chunk-contiguous input (one big-descriptor DMA per chunk).

Pass A tile [128, 2*6WP]: rows 0-23 G(0,0) | 24-31 rider q0-7 | 32-55 G(1,0)
  | 56-63 rider q8-15 | 64-87 G(2,0) | 88-95 rider q16-23 | 96-119 G(0,1).
Pass B tile [128, 2*4WP]: 0-23 G(1,1) | 32-55 G(2,1) | 64-87 G(0,2)
  | 96-119 G(1,2); gaps zero.
Group (dh,dw) partition q=3j+ic holds x_pad[ic, rs+u+dh+32j, v+dw] at u*WP+v.
Copies (src/dst partition bases all 32-aligned):
  C1 G10<-G00+WP, C2 G20<-G00+2WP, C3 G01<-G00+1e,
  C4 B[0:56]<-A[32:88]+1e, C5 B[64:120]<-A[0:56]+2e.
"""

import numpy as np

from concourse.ap import AP
import concourse.bass as bass
import concourse.mybir as mybir
import concourse.tile as tile
from concourse import bacc
from concourse.bass_utils import run_bass_kernel_spmd

IC, OC, KH, KW = 3, 16, 3, 3
H = W = 2048
N_CORES = 8
RPC = H // N_CORES          # 256
HP = RPC + 2                # 258
WP = W + 2                  # 2050

NB = 8                      # bands
BR = RPC // NB              # 32 rows per band
S = 4                       # rows per chunk
NCHUNK = BR // S            # 8
NWT = W // 512              # 4

PA = 6 * WP                 # pass-A half pitch
PB = 4 * WP                 # pass-B half pitch

F32 = mybir.dt.float32
FP16 = mybir.dt.float16
DT = FP16

# lhsT row maps: (row_start, (dh,dw), q_start, q_count)
MAP_A = [(0, (0, 0), 0, 24), (24, (2, 2), 0, 8), (32, (1, 0), 0, 24),
         (56, (2, 2), 8, 8), (64, (2, 0), 0, 24), (88, (2, 2), 16, 8),
         (96, (0, 1), 0, 24)]
MAP_B = [(0, (1, 1), 0, 24), (32, (2, 1), 0, 24), (64, (0, 2), 0, 24),
         (96, (1, 2), 0, 24)]


def build_nc() -> bass.Bass:
    nc = bacc.Bacc("TRN2", target_bir_lowering=False, debug=False)
    xs = nc.dram_tensor("xs", [NCHUNK, 24, 6 * WP], DT, kind="ExternalInput")
    xr = nc.dram_tensor("xr", [NCHUNK, 24, 4 * WP], DT, kind="ExternalInput")
    wa = nc.dram_tensor("wa", [128, 128], DT, kind="ExternalInput")
    wb = nc.dram_tensor("wb", [128, 128], DT, kind="ExternalInput")
    out = nc.dram_tensor("out", [OC, RPC, W], DT, kind="ExternalOutput")
    out_flat = out[:, :, :]

    def store_ap(r0):
        # dims (j, oc, t, w): element = out[oc, 32j + r0 + t, w]
        return AP(
            out_flat.tensor,
            r0 * W,
            [(BR * W, NB), (RPC * W, OC), (W, S), (1, W)],
        )

    with tile.TileContext(nc) as tc:
        with (
            tc.tile_pool(name="wpool", bufs=1) as wpool,
            tc.tile_pool(name="slaba", bufs=1) as slaba_pool,
            tc.tile_pool(name="slabb", bufs=1) as slabb_pool,
            tc.tile_pool(name="stgout", bufs=2) as stgout_pool,
            tc.tile_pool(name="psum", bufs=2, space="PSUM") as psum_pool,
        ):
            wa_sb = wpool.tile([128, 128], DT)
            wb_sb = wpool.tile([128, 128], DT)
            nc.sync.dma_start(out=wa_sb[:, :], in_=wa[:, :])
            nc.sync.dma_start(out=wb_sb[:, :], in_=wb[:, :])

            A = slaba_pool.tile([128, 2 * PA], DT)
            B = slabb_pool.tile([128, 2 * PB], DT)
            # zero once: gap tails never rewritten stay finite-zero forever
            nc.vector.memset(A[:, :], 0.0)
            nc.gpsimd.memset(B[:, :], 0.0)

            for kc in range(NCHUNK):
                h = kc % 2
                a0 = h * PA
                b0 = h * PB
                # G00: 6 raw rows (one 24.6KB descriptor per partition)
                nc.sync.dma_start(out=A[0:24, a0 : a0 + 6 * WP], in_=xs[kc, :, :])
                # rider G22 into pass-A gaps
                nc.sync.dma_start(
                    out=A[24:32, a0 : a0 + 4 * WP], in_=xr[kc, 0:8, :]
                )
                nc.sync.dma_start(
                    out=A[56:64, a0 : a0 + 4 * WP], in_=xr[kc, 8:16, :]
                )
                nc.sync.dma_start(
                    out=A[88:96, a0 : a0 + 4 * WP], in_=xr[kc, 16:24, :]
                )
                # shift chain
                nc.scalar.copy(
                    out=A[32:56, a0 : a0 + 4 * WP],
                    in_=A[0:24, a0 + WP : a0 + 5 * WP],
                )
                nc.vector.tensor_copy(
                    out=A[64:88, a0 : a0 + 4 * WP],
                    in_=A[0:24, a0 + 2 * WP : a0 + 6 * WP],
                )
                nc.scalar.copy(
                    out=A[96:120, a0 : a0 + 4 * WP],
                    in_=A[0:24, a0 + 1 : a0 + 4 * WP + 1],
                )
                nc.vector.tensor_copy(
                    out=B[0:24, b0 : b0 + 4 * WP],
                    in_=A[32:56, a0 + 1 : a0 + 4 * WP + 1],
                )
                nc.vector.tensor_copy(
                    out=B[32:56, b0 : b0 + 4 * WP],
                    in_=A[64:88, a0 + 1 : a0 + 4 * WP + 1],
                )
                nc.vector.tensor_copy(
                    out=B[64:120, b0 : b0 + 4 * WP],
                    in_=A[0:56, a0 + 2 : a0 + 4 * WP + 2],
                )

                stg = stgout_pool.tile([128, S * W], DT, tag="stg")
                for s in range(S):
                    ps = psum_pool.tile([128, W], F32, tag="ps")
                    for wt in range(NWT):
                        nc.tensor.matmul(
                            out=ps[:, wt * 512 : (wt + 1) * 512],
                            lhsT=wa_sb[:, :],
                            rhs=A[
                                :,
                                a0 + s * WP + wt * 512 : a0
                                + s * WP
                                + wt * 512
                                + 512,
                            ],
                            start=True,
                            stop=False,
                        )
                    for wt in range(NWT):
                        nc.tensor.matmul(
                            out=ps[:, wt * 512 : (wt + 1) * 512],
                            lhsT=wb_sb[:, :],
                            rhs=B[
                                :,
                                b0 + s * WP + wt * 512 : b0
                                + s * WP
                                + wt * 512
                                + 512,
                            ],
                            start=False,
                            stop=True,
                        )
                    if s % 2 == 0:
                        nc.scalar.copy(out=stg[:, s * W : (s + 1) * W], in_=ps[:, :])
                    else:
                        nc.vector.tensor_copy(
                            out=stg[:, s * W : (s + 1) * W], in_=ps[:, :]
                        )
                deng = nc.scalar if kc % 2 == 0 else nc.gpsimd
                deng.dma_start(out=store_ap(S * kc), in_=stg[:, :])

    nc.compile()
    return nc


def make_weights(kernel: np.ndarray):
    """kernel [OC, IC, KH, KW] -> lhsT [128,128] for passes A and B."""
    def mk(rowmap):
        wd = np.zeros((128, 128), np.float32)
        for r0, (dh, dw), q0, qn in rowmap:
            for qq in range(qn):
                j, ic = divmod(q0 + qq, 3)
                wd[r0 + qq, 16 * j : 16 * j + OC] = kernel[:, ic, dh, dw]
        return np.ascontiguousarray(wd.astype(np.float16))

    return mk(MAP_A), mk(MAP_B)


def make_xs(x_pad: np.ndarray, c: int) -> np.ndarray:
    """xs[kc, 3j+ic, u*WP+v] = x_pad[ic, c*RPC + 4kc + 32j + u, v], u in [0,6)."""
    xsl = x_pad[:, c * RPC : c * RPC + HP, :]
    kcs = np.arange(NCHUNK)[:, None, None]
    js = np.arange(NB)[None, :, None]
    us = np.arange(6)[None, None, :]
    rows = S * kcs + BR * js + us                     # [8, 8, 6]
    g = xsl[:, rows, :]                               # [IC, 8, 8, 6, WP]
    g = g.transpose(1, 2, 0, 3, 4)                    # [kc, j, ic, u, WP]
    return np.ascontiguousarray(g.reshape(NCHUNK, 24, 6 * WP))


def make_xr(x_pad: np.ndarray, c: int) -> np.ndarray:
    """Rider (dh=2,dw=2): xr[kc, 3j+ic, u*WP+v] = x_pad[ic, rs+u+2+32j, v+2],
    u in [0,4), last two columns of each slot zero."""
    xsl = x_pad[:, c * RPC : c * RPC + HP, :]
    kcs = np.arange(NCHUNK)[:, None, None]
    js = np.arange(NB)[None, :, None]
    us = np.arange(4)[None, None, :]
    rows = S * kcs + BR * js + us + 2                 # [8, 8, 4]
    g = xsl[:, rows, :]                               # [IC, 8, 8, 4, WP]
    g = g.transpose(1, 2, 0, 3, 4)                    # [kc, j, ic, 4, WP]
    out = np.zeros((NCHUNK, NB, IC, 4, WP), np.float16)
    out[:, :, :, :, : WP - 2] = g[:, :, :, :, 2:]
    return np.ascontiguousarray(out.reshape(NCHUNK, 24, 4 * WP))


_NC_CACHE = {}


def kernel(x: np.ndarray, kernel: np.ndarray) -> np.ndarray:
    assert x.shape == (IC, H, W) and kernel.shape == (OC, IC, KH, KW)
    x = np.ascontiguousarray(x, np.float32)
    kernel = np.ascontiguousarray(kernel, np.float32)

    if "nc" not in _NC_CACHE:
        _NC_CACHE["nc"] = build_nc()
    nc = _NC_CACHE["nc"]

    x_pad = np.zeros((IC, H + 2, W + 2), np.float16)
    x_pad[:, 1:-1, 1:-1] = x.astype(np.float16)
    wa, wb = make_weights(kernel)

    in_maps = []
    for c in range(N_CORES):
        in_maps.append(
            {"xs": make_xs(x_pad, c), "xr": make_xr(x_pad, c), "wa": wa, "wb": wb}
        )

    res = run_bass_kernel_spmd(nc, in_maps, core_ids=list(range(N_CORES)))
    outs = [res.results[c]["out"].astype(np.float32) for c in range(N_CORES)]
    return np.concatenate(outs, axis=1)
